# revision 9
# baseline (speedup 1.0000x reference)
"""LIF (leaky integrate-and-fire) recurrence kernel for Trainium2, 8 NeuronCores.

Problem: x (T=32, B=64, N=32768) f32.
    m[t] = tau*v[t-1] + x[t];  y[t] = (m[t] >= 1.0);  v[t] = m[t]*(1-y[t])
Output: y (32, 64, 32768) f32.

Sharding: data-parallel over batch. Core c handles x[:, 8c:8(c+1), :],
a (32, 262144)-element independent recurrence laid out [128, 2048] per step.

Per-core pipeline (bit-exact vs the f32 reference):
  DVE (two fused scalar_tensor_tensor ops per step -- the serial chain and
  the bottleneck; ~2.29us per [128,2048] op):
    m = (v * tau) + x_t            (in0 op0 scalar) op1 in1
    v = (m is_lt 1.0) * m          hard reset: v=m below threshold, else 0
  ACT (spike output, exact at the threshold):
    s = Sign(m - 1)                m-1 is exact (Sterbenz), s in {-1,0,+1}
    y = Sigmoid(1e4*s + 5e3)       saturates: s=-1 -> 0.0, s in {0,+1} -> 1.0

y is stored as uint8 (0/1 exact; host widens to f32), quartering write
traffic vs f32. The t=0 m-op is skipped (v0=0 so m0 = x0, the x tile is
consumed directly); the t=31 v-op is skipped (v[31] unused). A dummy
Sign/Sigmoid pair at the top preloads the ACT spline tables so the ~2.6us
of table loads hide under the first x DMA instead of delaying the first
spike. x loads are staged [1,3,4,...] timesteps on the sync HWDGE ring;
y stores flush every 2 timesteps on the scalar ring (the ACT sequencer has
slack for the 667ns DMA configs; putting them on the sync ring would block
x-load configs behind store semaphores).

Rejected after measurement: GPSIMD spike offload (shares an SBUF port with
2-port DVE ops; its tensor_scalar degrades 12x next to a busy DVE) and a
TensorE identity-matmul lane for m (fp32 matmul = 2 passes + per-pass
LDWEIGHTS = ~3.5ns/col/step, more than the DVE op it replaces).
"""

import sys

if "/opt/trn_rl_repo" not in sys.path:
    sys.path.insert(0, "/opt/trn_rl_repo")

import numpy as np

TAU = 0.5
V_TH = 1.0

N_CORES = 8
T, B, N = 32, 64, 32768
B_SH = B // N_CORES          # 8 batch rows per core
E = B_SH * N                 # 262144 elements per core per timestep
P = 128                      # SBUF partitions
F = E // P                   # 2048 f32 per partition per timestep

X_CHUNKS = [1, 1, 1, 1, 2, 2, 3, 4, 4, 4, 4, 4, 1]  # timesteps per x load
Y_CHUNKS = [2] * 15 + [1, 1]           # timesteps per y store (short drain)
LAST_DVE_STEPS = 1                     # spike on DVE for the last step(s)

_compiled = None


def _build():
    from concourse import bacc, tile, mybir

    f32 = mybir.dt.float32
    ydt = mybir.dt.uint8
    Sign = mybir.ActivationFunctionType.Sign
    Sigmoid = mybir.ActivationFunctionType.Sigmoid
    assert sum(X_CHUNKS) == T and sum(Y_CHUNKS) == T
    nc = bacc.Bacc("TRN2", debug=False, num_devices=N_CORES)
    x = nc.dram_tensor("x", [T, E], f32, kind="ExternalInput").ap()
    y = nc.dram_tensor("y", [T, E], ydt, kind="ExternalOutput").ap()

    # [t, p, f] views of DRAM
    x_r = x.rearrange("t (p f) -> t p f", p=P)
    y_r = y.rearrange("t (p f) -> t p f", p=P)

    with tile.TileContext(nc) as tc:
        with (
            tc.tile_pool(name="io", bufs=3) as io_pool,
            tc.tile_pool(name="state", bufs=1) as st_pool,
            tc.tile_pool(name="m", bufs=5) as m_pool,
            tc.tile_pool(name="s", bufs=3) as s_pool,
            tc.tile_pool(name="yp", bufs=3) as y_pool,
        ):
            # per-partition constants for the ACT affine args
            c_neg1 = st_pool.tile([P, 1], f32, tag="c_neg1")
            c_scale = st_pool.tile([P, 1], f32, tag="c_scale")
            c_bias = st_pool.tile([P, 1], f32, tag="c_bias")
            nc.gpsimd.memset(c_neg1[:], -V_TH)
            nc.gpsimd.memset(c_scale[:], 1.0e4)
            nc.gpsimd.memset(c_bias[:], 5.0e3)
            v = st_pool.tile([P, F], f32, tag="v")

            # warm the ACT spline tables while the first x chunk loads
            warm = st_pool.tile([P, 1], f32, tag="warm")
            nc.scalar.activation(out=warm[:], in_=c_neg1[:], func=Sign,
                                 bias=c_neg1[:], scale=1.0)
            nc.scalar.activation(out=warm[:], in_=warm[:], func=Sigmoid,
                                 bias=c_bias[:], scale=c_scale[:])

            # x loads: each chunk's DMA is gated on the previous chunk's
            # data (a 1-column gpsimd op reading chunk k writes into chunk
            # k+1's tile, so the WAR dep serializes the transfers). The 16
            # SDMA engines round-robin concurrent queues at packet
            # granularity, so racing configs split the bandwidth and the
            # urgently-needed first chunks arrive late; serialized, every
            # chunk rides at the full ~350 GB/s. Chunk sizes ramp [1,1,..4]
            # so arrival tracks consumption (4.58us/step vs 2.9us/step).
            x_tiles = {}          # t -> (tile, col offset)
            next_chunk = 0
            t_loaded = 0
            prev_xt = None        # previous chunk's tile (gate source)

            def load_chunk():
                nonlocal next_chunk, t_loaded, prev_xt
                n_t = X_CHUNKS[next_chunk]
                xt = io_pool.tile([P, 4 * F], f32, tag="x")
                if prev_xt is not None:
                    nc.gpsimd.tensor_scalar(
                        out=xt[:, :1], in0=prev_xt[:, :1], scalar1=0.0,
                        scalar2=None, op0=mybir.AluOpType.mult,
                    )
                nc.sync.dma_start(
                    out=xt[:, : n_t * F].rearrange("p (t f) -> p t f", t=n_t),
                    in_=x_r[t_loaded:t_loaded + n_t].rearrange("t p f -> p t f"),
                )
                for i in range(n_t):
                    x_tiles[t_loaded + i] = (xt, i * F)
                prev_xt = xt
                next_chunk += 1
                t_loaded += n_t

            load_chunk()
            y_t = None
            y_chunk_idx = 0
            y_off = 0  # timesteps into current y chunk
            for t in range(T):
                if t not in x_tiles:
                    load_chunk()
                if next_chunk < len(X_CHUNKS) and t == t_loaded - X_CHUNKS[next_chunk - 1]:
                    load_chunk()  # prefetch one chunk ahead
                xt, off = x_tiles.pop(t)
                xs = xt[:, off:off + F]
                n_yt = Y_CHUNKS[y_chunk_idx]
                if y_off == 0:
                    y_t = y_pool.tile([P, max(Y_CHUNKS) * F], ydt, tag="y")
                ys = y_t[:, y_off * F:(y_off + 1) * F]
                if t == 0:
                    # v0 = 0 so m0 = x0: consume the x tile as m directly
                    m = xs
                else:
                    mt = m_pool.tile([P, F], f32, tag="m")
                    # m = (v * tau) + x_t
                    nc.vector.scalar_tensor_tensor(
                        out=mt[:], in0=v[:], scalar=TAU, in1=xs,
                        op0=mybir.AluOpType.mult, op1=mybir.AluOpType.add,
                    )
                    m = mt[:]
                if t < T - 1:
                    # v = (m < vth) * m   (hard reset); v[31] is never used
                    nc.vector.scalar_tensor_tensor(
                        out=v[:], in0=m, scalar=V_TH, in1=m,
                        op0=mybir.AluOpType.is_lt, op1=mybir.AluOpType.mult,
                    )
                if t >= T - LAST_DVE_STEPS:
                    # tail steps: spike on DVE (fast 2x tensor_scalar) so the
                    # final stores don't wait for the ACT chain
                    nc.vector.tensor_scalar(
                        out=ys, in0=m, scalar1=V_TH, scalar2=1.0,
                        op0=mybir.AluOpType.is_ge, op1=mybir.AluOpType.mult,
                    )
                else:
                    # s = Sign(m - 1); y = Sigmoid(1e4*s + 5e3)
                    s = s_pool.tile([P, F], f32, tag="s")
                    nc.scalar.activation(
                        out=s[:], in_=m, func=Sign,
                        bias=c_neg1[:], scale=1.0,
                    )
                    nc.scalar.activation(
                        out=ys, in_=s[:], func=Sigmoid,
                        bias=c_bias[:], scale=c_scale[:],
                    )
                y_off += 1
                if y_off == n_yt:
                    nc.scalar.dma_start(
                        out=y_r[t - n_yt + 1:t + 1].rearrange("t p f -> p t f"),
                        in_=y_t[:, : n_yt * F].rearrange("p (t f) -> p t f", t=n_yt),
                    )
                    y_chunk_idx += 1
                    y_off = 0
    nc.compile()
    return nc


def _get_compiled():
    global _compiled
    if _compiled is None:
        _compiled = _build()
        # warm the NEFF (first execution pays ~20us of cold-start)
        import concourse.bass_utils as bass_utils

        z = [{"x": np.zeros((T, E), dtype=np.float32)} for _ in range(N_CORES)]
        bass_utils.run_bass_kernel_spmd(
            _compiled, z, core_ids=list(range(N_CORES))
        )
    return _compiled


def kernel(x: np.ndarray, _trace: bool = False):
    import concourse.bass_utils as bass_utils

    nc = _get_compiled()
    x = np.ascontiguousarray(x, dtype=np.float32)
    in_maps = [
        {"x": x[:, c * B_SH:(c + 1) * B_SH, :].reshape(T, E)}
        for c in range(N_CORES)
    ]
    res = bass_utils.run_bass_kernel_spmd(
        nc, in_maps, core_ids=list(range(N_CORES)), trace=_trace
    )
    y = np.empty((T, B, N), dtype=np.float32)
    for c in range(N_CORES):
        yc = res.results[c]["y"]
        if yc.dtype != np.float32:
            yc = yc.astype(np.float32)  # uint8 0/1 -> f32, exact
        y[:, c * B_SH:(c + 1) * B_SH, :] = yc.reshape(T, B_SH, N)
    if _trace:
        return y, res
    return y


# revision 11
# speedup vs baseline: 1.2509x; 1.2509x over previous
"""LIF (leaky integrate-and-fire) recurrence kernel for Trainium2, 8 NeuronCores.

Problem: x (T=32, B=64, N=32768) f32.
    m[t] = tau*v[t-1] + x[t];  y[t] = (m[t] >= 1.0);  v[t] = m[t]*(1-y[t])
Output: y (32, 64, 32768) f32.

Sharding: data-parallel over batch. Core c handles x[:, 8c:8(c+1), :],
a (32, 262144)-element independent recurrence laid out [128, 2048] per step.

Per-core pipeline (bit-exact vs the f32 reference):
  DVE (two fused scalar_tensor_tensor ops per step -- the serial chain and
  the bottleneck; ~2.29us per [128,2048] op):
    m = (v * tau) + x_t            (in0 op0 scalar) op1 in1
    v = (m is_lt 1.0) * m          hard reset: v=m below threshold, else 0
  ACT (spike output, exact at the threshold):
    s = Sign(m - 1)                m-1 is exact (Sterbenz), s in {-1,0,+1}
    y = Sigmoid(1e4*s + 5e3)       saturates: s=-1 -> 0.0, s in {0,+1} -> 1.0

y is stored as uint8 (0/1 exact; host widens to f32), quartering write
traffic vs f32. The t=0 m-op is skipped (v0=0 so m0 = x0, the x tile is
consumed directly); the t=31 v-op is skipped (v[31] unused). A dummy
Sign/Sigmoid pair at the top preloads the ACT spline tables so the ~2.6us
of table loads hide under the first x DMA instead of delaying the first
spike. x loads are staged [1,3,4,...] timesteps on the sync HWDGE ring;
y stores flush every 2 timesteps on the scalar ring (the ACT sequencer has
slack for the 667ns DMA configs; putting them on the sync ring would block
x-load configs behind store semaphores).

Rejected after measurement: GPSIMD spike offload (shares an SBUF port with
2-port DVE ops; its tensor_scalar degrades 12x next to a busy DVE) and a
TensorE identity-matmul lane for m (fp32 matmul = 2 passes + per-pass
LDWEIGHTS = ~3.5ns/col/step, more than the DVE op it replaces).
"""

import sys

if "/opt/trn_rl_repo" not in sys.path:
    sys.path.insert(0, "/opt/trn_rl_repo")

import numpy as np

TAU = 0.5
V_TH = 1.0

N_CORES = 8
T, B, N = 32, 64, 32768
B_SH = B // N_CORES          # 8 batch rows per core
E = B_SH * N                 # 262144 elements per core per timestep
P = 128                      # SBUF partitions
F = E // P                   # 2048 f32 per partition per timestep

X_CHUNKS = [1, 3] + [4] * 7            # timesteps per x load (fast fill)
Y_CHUNKS = [2] * 15 + [1, 1]           # timesteps per y store (short drain)
LAST_DVE_STEPS = 1                     # spike on DVE for the last step(s)

_compiled = None


def _build():
    from concourse import bacc, tile, mybir

    f32 = mybir.dt.float32
    ydt = mybir.dt.uint8
    Sign = mybir.ActivationFunctionType.Sign
    Sigmoid = mybir.ActivationFunctionType.Sigmoid
    assert sum(X_CHUNKS) == T and sum(Y_CHUNKS) == T
    nc = bacc.Bacc("TRN2", debug=False, num_devices=N_CORES)
    x = nc.dram_tensor("x", [T, E], f32, kind="ExternalInput").ap()
    y = nc.dram_tensor("y", [T, E], ydt, kind="ExternalOutput").ap()

    # [t, p, f] views of DRAM
    x_r = x.rearrange("t (p f) -> t p f", p=P)
    y_r = y.rearrange("t (p f) -> t p f", p=P)

    with tile.TileContext(nc) as tc:
        with (
            tc.tile_pool(name="io", bufs=3) as io_pool,
            tc.tile_pool(name="state", bufs=1) as st_pool,
            tc.tile_pool(name="m", bufs=5) as m_pool,
            tc.tile_pool(name="s", bufs=3) as s_pool,
            tc.tile_pool(name="yp", bufs=3) as y_pool,
        ):
            # per-partition constants for the ACT affine args
            c_neg1 = st_pool.tile([P, 1], f32, tag="c_neg1")
            c_scale = st_pool.tile([P, 1], f32, tag="c_scale")
            c_bias = st_pool.tile([P, 1], f32, tag="c_bias")
            nc.gpsimd.memset(c_neg1[:], -V_TH)
            nc.gpsimd.memset(c_scale[:], 1.0e4)
            nc.gpsimd.memset(c_bias[:], 5.0e3)
            v = st_pool.tile([P, F], f32, tag="v")

            # warm the ACT spline tables while the first x chunk loads
            warm = st_pool.tile([P, 1], f32, tag="warm")
            nc.scalar.activation(out=warm[:], in_=c_neg1[:], func=Sign,
                                 bias=c_neg1[:], scale=1.0)
            nc.scalar.activation(out=warm[:], in_=warm[:], func=Sigmoid,
                                 bias=c_bias[:], scale=c_scale[:])

            # issue x loads lazily, two chunks ahead of consumption
            x_tiles = {}          # t -> (tile, col offset)
            next_chunk = 0
            t_loaded = 0

            def load_chunk():
                nonlocal next_chunk, t_loaded
                n_t = X_CHUNKS[next_chunk]
                xt = io_pool.tile([P, 4 * F], f32, tag="x")
                nc.sync.dma_start(
                    out=xt[:, : n_t * F].rearrange("p (t f) -> p t f", t=n_t),
                    in_=x_r[t_loaded:t_loaded + n_t].rearrange("t p f -> p t f"),
                )
                for i in range(n_t):
                    x_tiles[t_loaded + i] = (xt, i * F)
                next_chunk += 1
                t_loaded += n_t

            load_chunk()
            y_t = None
            y_chunk_idx = 0
            y_off = 0  # timesteps into current y chunk
            for t in range(T):
                if t not in x_tiles:
                    load_chunk()
                if next_chunk < len(X_CHUNKS) and t == t_loaded - X_CHUNKS[next_chunk - 1]:
                    load_chunk()  # prefetch one chunk ahead
                xt, off = x_tiles.pop(t)
                xs = xt[:, off:off + F]
                n_yt = Y_CHUNKS[y_chunk_idx]
                if y_off == 0:
                    y_t = y_pool.tile([P, max(Y_CHUNKS) * F], ydt, tag="y")
                ys = y_t[:, y_off * F:(y_off + 1) * F]
                if t == 0:
                    # v0 = 0 so m0 = x0: consume the x tile as m directly
                    m = xs
                else:
                    mt = m_pool.tile([P, F], f32, tag="m")
                    # m = (v * tau) + x_t
                    nc.vector.scalar_tensor_tensor(
                        out=mt[:], in0=v[:], scalar=TAU, in1=xs,
                        op0=mybir.AluOpType.mult, op1=mybir.AluOpType.add,
                    )
                    m = mt[:]
                if t < T - 1:
                    # v = (m < vth) * m   (hard reset); v[31] is never used
                    nc.vector.scalar_tensor_tensor(
                        out=v[:], in0=m, scalar=V_TH, in1=m,
                        op0=mybir.AluOpType.is_lt, op1=mybir.AluOpType.mult,
                    )
                if t >= T - LAST_DVE_STEPS:
                    # tail steps: spike on DVE (fast 2x tensor_scalar) so the
                    # final stores don't wait for the ACT chain
                    nc.vector.tensor_scalar(
                        out=ys, in0=m, scalar1=V_TH, scalar2=1.0,
                        op0=mybir.AluOpType.is_ge, op1=mybir.AluOpType.mult,
                    )
                else:
                    # s = Sign(m - 1); y = Sigmoid(1e4*s + 5e3)
                    s = s_pool.tile([P, F], f32, tag="s")
                    nc.scalar.activation(
                        out=s[:], in_=m, func=Sign,
                        bias=c_neg1[:], scale=1.0,
                    )
                    nc.scalar.activation(
                        out=ys, in_=s[:], func=Sigmoid,
                        bias=c_bias[:], scale=c_scale[:],
                    )
                y_off += 1
                if y_off == n_yt:
                    nc.scalar.dma_start(
                        out=y_r[t - n_yt + 1:t + 1].rearrange("t p f -> p t f"),
                        in_=y_t[:, : n_yt * F].rearrange("p (t f) -> p t f", t=n_yt),
                    )
                    y_chunk_idx += 1
                    y_off = 0
    nc.compile()
    return nc


def _get_compiled():
    global _compiled
    if _compiled is None:
        _compiled = _build()
        # warm the NEFF (first execution pays ~20us of cold-start)
        import concourse.bass_utils as bass_utils

        z = [{"x": np.zeros((T, E), dtype=np.float32)} for _ in range(N_CORES)]
        bass_utils.run_bass_kernel_spmd(
            _compiled, z, core_ids=list(range(N_CORES))
        )
    return _compiled


def kernel(x: np.ndarray, _trace: bool = False):
    import concourse.bass_utils as bass_utils

    nc = _get_compiled()
    x = np.ascontiguousarray(x, dtype=np.float32)
    in_maps = [
        {"x": x[:, c * B_SH:(c + 1) * B_SH, :].reshape(T, E)}
        for c in range(N_CORES)
    ]
    res = bass_utils.run_bass_kernel_spmd(
        nc, in_maps, core_ids=list(range(N_CORES)), trace=_trace
    )
    y = np.empty((T, B, N), dtype=np.float32)
    for c in range(N_CORES):
        yc = res.results[c]["y"]
        if yc.dtype != np.float32:
            yc = yc.astype(np.float32)  # uint8 0/1 -> f32, exact
        y[:, c * B_SH:(c + 1) * B_SH, :] = yc.reshape(T, B_SH, N)
    if _trace:
        return y, res
    return y


# revision 17
# speedup vs baseline: 1.3198x; 1.0551x over previous
"""LIF (leaky integrate-and-fire) recurrence kernel for Trainium2, 8 NeuronCores.

Problem: x (T=32, B=64, N=32768) f32.
    m[t] = tau*v[t-1] + x[t];  y[t] = (m[t] >= 1.0);  v[t] = m[t]*(1-y[t])
Output: y (32, 64, 32768) f32.

Sharding: data-parallel over batch. Core c handles x[:, 8c:8(c+1), :],
a (32, 262144)-element independent recurrence laid out [128, 2048] per step.

Per-core pipeline (bit-exact vs the f32 reference):
  DVE (two fused scalar_tensor_tensor ops per step -- the serial chain and
  the bottleneck; ~2.29us per [128,2048] op):
    m = (v * tau) + x_t            (in0 op0 scalar) op1 in1
    v = (m is_lt 1.0) * m          hard reset: v=m below threshold, else 0
  ACT (spike output, exact at the threshold):
    s = Sign(m - 1)                m-1 is exact (Sterbenz), s in {-1,0,+1}
    y = Sigmoid(1e4*s + 5e3)       saturates: s=-1 -> 0.0, s in {0,+1} -> 1.0

y is stored as uint8 (0/1 exact; host widens to f32), quartering write
traffic vs f32. The t=0 m-op is skipped (v0=0 so m0 = x0, the x tile is
consumed directly); the t=31 v-op is skipped (v[31] unused). A dummy
Sign/Sigmoid pair at the top preloads the ACT spline tables so the ~2.6us
of table loads hide under the first x DMA instead of delaying the first
spike. x loads are one per-timestep DMA on the sync HWDGE ring, issued in
consumption order 8 deep (equal-size queues complete in issue order under
the SDMA round-robin, so each step waits only on its own 1 MB semaphore);
y stores flush every 2 timesteps on the scalar ring (the ACT sequencer has
slack for the 667ns DMA configs; putting them on the sync ring would block
x-load configs behind store semaphores).

Rejected after measurement: GPSIMD spike offload (shares an SBUF port with
2-port DVE ops; its tensor_scalar degrades 12x next to a busy DVE) and a
TensorE identity-matmul lane for m (fp32 matmul = 2 passes + per-pass
LDWEIGHTS = ~3.5ns/col/step, more than the DVE op it replaces).
"""

import sys

if "/opt/trn_rl_repo" not in sys.path:
    sys.path.insert(0, "/opt/trn_rl_repo")

import numpy as np

TAU = 0.5
V_TH = 1.0

N_CORES = 8
T, B, N = 32, 64, 32768
B_SH = B // N_CORES          # 8 batch rows per core
E = B_SH * N                 # 262144 elements per core per timestep
P = 128                      # SBUF partitions
F = E // P                   # 2048 f32 per partition per timestep

X_AHEAD = 7                            # x prefetch depth (per-step DMAs)
Y_CHUNKS = [2] * 15 + [1, 1]           # timesteps per y store (short drain)
LAST_DVE_STEPS = 1                     # spike on DVE for the last step(s)

_compiled = None


def _build():
    from concourse import bacc, tile, mybir

    f32 = mybir.dt.float32
    ydt = mybir.dt.uint8
    Sign = mybir.ActivationFunctionType.Sign
    Sigmoid = mybir.ActivationFunctionType.Sigmoid
    assert sum(Y_CHUNKS) == T
    nc = bacc.Bacc("TRN2", debug=False, num_devices=N_CORES)
    x = nc.dram_tensor("x", [T, E], f32, kind="ExternalInput").ap()
    y = nc.dram_tensor("y", [T, E], ydt, kind="ExternalOutput").ap()

    # [t, p, f] views of DRAM
    x_r = x.rearrange("t (p f) -> t p f", p=P)
    y_r = y.rearrange("t (p f) -> t p f", p=P)

    with tile.TileContext(nc) as tc:
        with (
            tc.tile_pool(name="io", bufs=X_AHEAD + 2) as io_pool,
            tc.tile_pool(name="state", bufs=1) as st_pool,
            tc.tile_pool(name="m", bufs=5) as m_pool,
            tc.tile_pool(name="s", bufs=3) as s_pool,
            tc.tile_pool(name="yp", bufs=3) as y_pool,
        ):
            # per-partition constants for the ACT affine args
            c_neg1 = st_pool.tile([P, 1], f32, tag="c_neg1")
            c_scale = st_pool.tile([P, 1], f32, tag="c_scale")
            c_bias = st_pool.tile([P, 1], f32, tag="c_bias")
            nc.gpsimd.memset(c_neg1[:], -V_TH)
            nc.gpsimd.memset(c_scale[:], 1.0e4)
            nc.gpsimd.memset(c_bias[:], 5.0e3)
            v = st_pool.tile([P, F], f32, tag="v")

            # warm the ACT spline tables while the first x chunk loads
            warm = st_pool.tile([P, 1], f32, tag="warm")
            nc.scalar.activation(out=warm[:], in_=c_neg1[:], func=Sign,
                                 bias=c_neg1[:], scale=1.0)
            nc.scalar.activation(out=warm[:], in_=warm[:], func=Sigmoid,
                                 bias=c_bias[:], scale=c_scale[:])

            # x loads: one DMA config per timestep, issued in consumption
            # order, X_AHEAD steps deep. Equal-size queues issued in order
            # complete in order under the SDMA engines' fair round-robin
            # (~2.9us apart at full bandwidth), and each step's compute
            # waits only on its own 1 MB completion semaphore -- multi-step
            # chunks used to stall the fill ~8us because a chunk's sem only
            # fired after later chunks' interleaved packets drained.
            x_tiles = {}          # t -> tile
            t_loaded = 0

            def load_step():
                nonlocal t_loaded
                xt = io_pool.tile([P, F], f32, tag="x")
                nc.sync.dma_start(out=xt[:], in_=x_r[t_loaded])
                x_tiles[t_loaded] = xt
                t_loaded += 1

            for _ in range(X_AHEAD + 1):
                load_step()
            y_t = None
            y_chunk_idx = 0
            y_off = 0  # timesteps into current y chunk
            for t in range(T):
                if t_loaded < T:
                    load_step()   # keep the prefetch window full
                xs = x_tiles.pop(t)[:]
                n_yt = Y_CHUNKS[y_chunk_idx]
                if y_off == 0:
                    y_t = y_pool.tile([P, max(Y_CHUNKS) * F], ydt, tag="y")
                ys = y_t[:, y_off * F:(y_off + 1) * F]
                if t == 0:
                    # v0 = 0 so m0 = x0: consume the x tile as m directly
                    m = xs
                else:
                    mt = m_pool.tile([P, F], f32, tag="m")
                    # m = (v * tau) + x_t
                    nc.vector.scalar_tensor_tensor(
                        out=mt[:], in0=v[:], scalar=TAU, in1=xs,
                        op0=mybir.AluOpType.mult, op1=mybir.AluOpType.add,
                    )
                    m = mt[:]
                if t < T - 1:
                    # v = (m < vth) * m   (hard reset); v[31] is never used
                    nc.vector.scalar_tensor_tensor(
                        out=v[:], in0=m, scalar=V_TH, in1=m,
                        op0=mybir.AluOpType.is_lt, op1=mybir.AluOpType.mult,
                    )
                if t >= T - LAST_DVE_STEPS:
                    # tail steps: spike on DVE (fast 2x tensor_scalar) so the
                    # final stores don't wait for the ACT chain
                    nc.vector.tensor_scalar(
                        out=ys, in0=m, scalar1=V_TH, scalar2=1.0,
                        op0=mybir.AluOpType.is_ge, op1=mybir.AluOpType.mult,
                    )
                else:
                    # s = Sign(m - 1); y = Sigmoid(1e4*s + 5e3)
                    s = s_pool.tile([P, F], f32, tag="s")
                    nc.scalar.activation(
                        out=s[:], in_=m, func=Sign,
                        bias=c_neg1[:], scale=1.0,
                    )
                    nc.scalar.activation(
                        out=ys, in_=s[:], func=Sigmoid,
                        bias=c_bias[:], scale=c_scale[:],
                    )
                y_off += 1
                if y_off == n_yt:
                    nc.scalar.dma_start(
                        out=y_r[t - n_yt + 1:t + 1].rearrange("t p f -> p t f"),
                        in_=y_t[:, : n_yt * F].rearrange("p (t f) -> p t f", t=n_yt),
                    )
                    y_chunk_idx += 1
                    y_off = 0
    nc.compile()
    return nc


def _get_compiled():
    global _compiled
    if _compiled is None:
        _compiled = _build()
        # warm the NEFF (first execution pays ~20us of cold-start)
        import concourse.bass_utils as bass_utils

        z = [{"x": np.zeros((T, E), dtype=np.float32)} for _ in range(N_CORES)]
        bass_utils.run_bass_kernel_spmd(
            _compiled, z, core_ids=list(range(N_CORES))
        )
    return _compiled


def kernel(x: np.ndarray, _trace: bool = False):
    import concourse.bass_utils as bass_utils

    nc = _get_compiled()
    x = np.ascontiguousarray(x, dtype=np.float32)
    in_maps = [
        {"x": x[:, c * B_SH:(c + 1) * B_SH, :].reshape(T, E)}
        for c in range(N_CORES)
    ]
    res = bass_utils.run_bass_kernel_spmd(
        nc, in_maps, core_ids=list(range(N_CORES)), trace=_trace
    )
    y = np.empty((T, B, N), dtype=np.float32)
    for c in range(N_CORES):
        yc = res.results[c]["y"]
        if yc.dtype != np.float32:
            yc = yc.astype(np.float32)  # uint8 0/1 -> f32, exact
        y[:, c * B_SH:(c + 1) * B_SH, :] = yc.reshape(T, B_SH, N)
    if _trace:
        return y, res
    return y
